# revision 14
# baseline (speedup 1.0000x reference)
"""Trainium2 Bass kernel for nn_CrossAttentionBlock (GroupNorm + 1x1-conv Q +
cross-attention over cond + output projection + residual).

Full-input contract: kernel(**inputs) takes the complete unsharded inputs and
returns the full [16, 512, 64, 64] float32 output.  Internally shards
data-parallel over batch across 8 NeuronCores (2 batches per core), runs one
SPMD Bass/Tile kernel via run_bass_kernel_spmd, and concatenates the results.

v2 layout strategy (per core, per batch, channels-first [C, HW]):
  - GroupNorm stats via DVE bn_stats/bn_aggr (one pass, mean+var), group
    reduce/scatter via tiny indicator matmuls.
  - kq fusion: kq_h = Wq_h^T (k_h + kb) computed once per batch (bf16 PE,
    K=64), quantized to fp8e4m3 (x64).  The per-chunk Q projection and its
    PSUM drain disappear; logits^T = kq_h^T xn via 2 fp8 DoubleRow matmuls
    per head (K folded 512 -> 2x(128x2)).  qb^T k and the attention scale
    fold into the exp bias/scale (per-partition cols).
  - xn = (x*sc + tc) -> fp8 on the GpSimd (Pool) engine, pair-tile layout
    [128, 2, 512] matching DoubleRow rhs.
  - softmax denominators: ones[77,64] matmuls replicate per-head sums over
    the head's 64 output channels; reciprocal_approx_fast (DVE) + one
    tensor_mul per head-pair normalizes straight into fp8 pair tiles.
  - out proj: fp8 DoubleRow with pwT x64; PSUM drain on ACT fuses the
    2^-8 descale + proj bias; residual add (x from SBUF-resident xb) on
    Pool; one DMA per chunk via a [B, 128, 4, HW] permuted DRAM layout
    (host unpermutes).
Weights are transposed/cast/packed on the host (layout prep only).
"""

import sys

for _p in ("/opt/trn_rl_repo",):
    if _p not in sys.path:
        sys.path.append(_p)

from contextlib import ExitStack

import numpy as np
import ml_dtypes

import concourse.bacc as bacc
import concourse.tile as tile
from concourse import mybir
from concourse.bass_utils import run_bass_kernel_spmd

BF16 = ml_dtypes.bfloat16
F8 = ml_dtypes.float8_e4m3

N_CORES = 8
B, C, H, W = 16, 512, 64, 64
HW = H * W                      # 4096
L, CD = 77, 768
NH, HD = 8, 64                  # heads, head dim
NG, GS = 32, 16                 # groups, channels per group
EPS = 1e-6
B_LOC = B // N_CORES            # 2
NT = C // 128                   # 4 channel tiles
NJ = NT // 2                    # 2 DoubleRow k-pair tiles
KT = CD // 128                  # 6 cond-dim tiles
CH = 512                        # hw chunk
NCH = HW // CH                  # 8
HLF = HW // 2                   # 2048 (x half-tile width)
GPT = 128 // GS                 # 8 groups per 128-channel tile
WS = 64.0                       # fp8 weight scale (qk and proj paths)
VS = 4.0                        # v scale (folded into vwT/vb on host)
LP = 80                         # L padded (DoubleRow lhsT alignment)


def _build_nc(nch=NCH, reps=1):
    f32 = mybir.dt.float32
    bf16 = mybir.dt.bfloat16
    f8 = mybir.dt.float8e4
    nc = bacc.Bacc("TRN2", target_bir_lowering=False, debug=False)

    x_d = nc.dram_tensor("x", [B_LOC, C, HW], f32, kind="ExternalInput").ap()
    condT_d = nc.dram_tensor("condT", [B_LOC, CD, L], bf16,
                             kind="ExternalInput").ap()
    qw_d = nc.dram_tensor("qw", [C, C], bf16, kind="ExternalInput").ap()
    kwT_d = nc.dram_tensor("kwT", [CD, C], bf16, kind="ExternalInput").ap()
    vwT_d = nc.dram_tensor("vwT", [CD, C], bf16, kind="ExternalInput").ap()
    pwdr_d = nc.dram_tensor("pwdr", [NJ, 128, 2, C], f8,
                            kind="ExternalInput").ap()
    # colv cols: 0-3 gamma, 4-7 beta, 8-11 qb, 12-15 kb, 16-19 pb (per c-tile)
    colv_d = nc.dram_tensor("colv", [128, 20], f32, kind="ExternalInput").ap()
    vb_d = nc.dram_tensor("vb", [1, C], f32, kind="ExternalInput").ap()
    scale_d = nc.dram_tensor("scale", [1, 1], f32, kind="ExternalInput").ap()
    g16_d = nc.dram_tensor("g16", [128, GPT], f32, kind="ExternalInput").ap()
    g16T_d = nc.dram_tensor("g16T", [GPT, 128], f32, kind="ExternalInput").ap()
    out_d = nc.dram_tensor("out", [B_LOC, 128, NT, HW], f32,
                           kind="ExternalOutput").ap()

    AO = mybir.AluOpType
    AF = mybir.ActivationFunctionType

    with tile.TileContext(nc) as tc, ExitStack() as ctx, \
            nc.allow_low_precision("fp8/bf16 attention pipeline"):
        # --- pools ---
        wp = ctx.enter_context(tc.tile_pool(name="weights", bufs=1))
        sbx = ctx.enter_context(tc.tile_pool(name="xbuf", bufs=1))
        sbb = ctx.enter_context(tc.tile_pool(name="batch", bufs=2))
        sb2 = ctx.enter_context(tc.tile_pool(name="work2", bufs=2))
        sb3 = ctx.enter_context(tc.tile_pool(name="work3", bufs=2))
        ps_qk = ctx.enter_context(tc.tile_pool(name="ps_qk", bufs=2,
                                               space="PSUM"))
        ps_sm = ctx.enter_context(tc.tile_pool(name="ps_sm", bufs=2,
                                               space="PSUM"))
        ps_av = ctx.enter_context(tc.tile_pool(name="ps_av", bufs=2,
                                               space="PSUM"))
        ps_o = ctx.enter_context(tc.tile_pool(name="ps_o", bufs=2,
                                              space="PSUM"))

        # --- persistent weights/constants ---
        qw_sb = [wp.tile([128, C], bf16, tag=f"qw{t}", name=f"qw{t}")
                 for t in range(NT)]
        kwT = [wp.tile([128, C], bf16, tag=f"kwT{j}", name=f"kwT{j}")
               for j in range(KT)]
        vwT = [wp.tile([128, C], bf16, tag=f"vwT{j}", name=f"vwT{j}")
               for j in range(KT)]
        pwdr = [wp.tile([128, 2, C], f8, tag=f"pwdr{j}", name=f"pwdr{j}")
                for j in range(NJ)]
        for t in range(NT):
            nc.sync.dma_start(qw_sb[t][:], qw_d[128 * t:128 * (t + 1), :])
        for j in range(KT):
            nc.sync.dma_start(kwT[j][:], kwT_d[128 * j:128 * (j + 1), :])
            nc.sync.dma_start(vwT[j][:], vwT_d[128 * j:128 * (j + 1), :])
        for j in range(NJ):
            nc.sync.dma_start(pwdr[j][:], pwdr_d[j])

        g16 = wp.tile([128, GPT], f32, tag="g16")
        nc.sync.dma_start(g16[:], g16_d[:, :])
        g16T = wp.tile([GPT, 128], f32, tag="g16T")
        nc.sync.dma_start(g16T[:], g16T_d[:, :])
        colv = wp.tile([128, 20], f32, tag="colv")
        nc.sync.dma_start(colv[:], colv_d[:, :])
        vb_row = wp.tile([1, C], f32, tag="vb_row")
        nc.sync.dma_start(vb_row[:], vb_d[:, :])
        s11 = wp.tile([1, 1], f32, tag="s11")
        nc.sync.dma_start(s11[:], scale_d[:, :])
        scale_col = wp.tile([128, 1], f32, tag="scale_col")
        nc.gpsimd.partition_broadcast(scale_col[:], s11[:])
        # exp scale: attention scale / WS (kq fp8 descale)
        se_col = wp.tile([128, 1], f32, tag="se_col")
        nc.vector.tensor_scalar_mul(se_col[:], scale_col[:], 1.0 / WS)
        ones77 = wp.tile([L, 64], bf16, tag="ones77")
        nc.gpsimd.memset(ones77[:], 1.0)
        # v bias (x VS on host) broadcast over the 77 cond rows
        vb_bc = wp.tile([L, C], f32, tag="vb_bc")
        nc.gpsimd.partition_broadcast(vb_bc[:], vb_row[:])
        # q bias in bf16 (rhs of the qb^T k matmuls)
        qb_bf = wp.tile([128, NT], bf16, tag="qb_bf")
        nc.vector.tensor_copy(qb_bf[:], colv[:, 8:12])

        rep_ctx = tc.For_i(0, reps, 1) if reps > 1 else None
        if rep_ctx is not None:
            rep_ctx.__enter__()
        for b in range(B_LOC):
            # ---------- load x (half tiles for finer cross-batch overlap) ----
            xb = [[sbx.tile([128, HLF], f32, tag=f"x{t}h{h}", name=f"x{t}h{h}")
                   for h in range(2)] for t in range(NT)]
            for t in range(NT):
                for h in range(2):
                    nc.sync.dma_start(
                        xb[t][h][:],
                        x_d[b, 128 * t:128 * (t + 1), HLF * h:HLF * (h + 1)])

            # ---------- groupnorm stats (bn_stats, one DVE pass) ----------
            st = sbb.tile([128, 2, 4, 6], f32, tag="st")
            mv = sbb.tile([128, NT, 2], f32, tag="mv")
            wk = sbb.tile([128, 8], f32, tag="wk")
            for t in range(NT):
                for h in range(2):
                    for q in range(4):
                        nc.vector.bn_stats(st[:, h, q, :],
                                           xb[t][h][:, CH * q:CH * (q + 1)])
                nc.vector.bn_aggr(mv[:, t, :], st[:])
                # wk cols: t = mean, 4+t = E[x^2] = var + mean^2
                nc.vector.tensor_copy(wk[:, t:t + 1], mv[:, t, 0:1])
                nc.vector.scalar_tensor_tensor(
                    wk[:, 4 + t:5 + t], mv[:, t, 0:1], mv[:, t, 0:1],
                    mv[:, t, 1:2], AO.mult, AO.add)
            gm = ps_sm.tile([GPT, 8], f32, tag="sm")
            nc.tensor.matmul(gm[:], g16[:], wk[:], start=True, stop=True)
            # group var/rsig; gw cols: 0-3 mu_g, 4-7 rsig_g
            gw = sbb.tile([GPT, 8], f32, tag="gw")
            gtmp = sbb.tile([GPT, 8], f32, tag="gtmp")
            nc.vector.tensor_copy(gw[:, 0:4], gm[:, 0:4])
            nc.vector.tensor_mul(gtmp[:, 0:4], gw[:, 0:4], gw[:, 0:4])
            nc.vector.tensor_sub(gtmp[:, 4:8], gm[:, 4:8], gtmp[:, 0:4])
            nc.vector.tensor_scalar_add(gtmp[:, 4:8], gtmp[:, 4:8], EPS)
            nc.scalar.sqrt(gtmp[:, 0:4], gtmp[:, 4:8])
            nc.vector.reciprocal_approx_accurate(gw[:, 4:8], gtmp[:, 0:4],
                                                 gtmp[:, 4:8])
            cst = ps_sm.tile([128, 8], f32, tag="sm")
            nc.tensor.matmul(cst[:], g16T[:], gw[:], start=True, stop=True)
            # scb cols: t = sc (gamma*rsig), 4+t = tc (beta - mu*sc)
            scb = sbb.tile([128, 8], f32, tag="scb")
            mtmp = sbb.tile([128, 4], f32, tag="mtmp")
            for t in range(NT):
                nc.vector.tensor_mul(scb[:, t:t + 1], cst[:, 4 + t:5 + t],
                                     colv[:, t:t + 1])
                nc.vector.tensor_mul(mtmp[:, t:t + 1], cst[:, t:t + 1],
                                     scb[:, t:t + 1])
                nc.vector.tensor_sub(scb[:, 4 + t:5 + t],
                                     colv[:, 4 + t:5 + t], mtmp[:, t:t + 1])

            # ---------- K/V projections from cond ----------
            cT = [sbb.tile([128, L], bf16, tag=f"cT{j}", name=f"cT{j}")
                  for j in range(KT)]
            for j in range(KT):
                nc.sync.dma_start(cT[j][:], condT_d[b, 128 * j:128 * (j + 1), :])
            kT = [sbb.tile([128, LP], bf16, tag=f"kT{t}", name=f"kT{t}")
                  for t in range(NT)]
            v_sb = sbb.tile([L, C], bf16, tag="v_sb")
            for t in range(NT):
                nc.gpsimd.memset(kT[t][:, L:LP], 0.0)
            for t in range(NT):
                cs = slice(128 * t, 128 * (t + 1))
                pk = ps_qk.tile([128, CH], f32, tag="qk")
                for j in range(KT):
                    nc.tensor.matmul(pk[:, 0:L], kwT[j][:, cs], cT[j][:],
                                     start=(j == 0), stop=(j == KT - 1))
                nc.scalar.activation(kT[t][:, 0:L], pk[:, 0:L], AF.Identity,
                                     bias=colv[:, 12 + t:13 + t])
                pv = ps_av.tile([128, CH], f32, tag="av")
                for j in range(KT):
                    nc.tensor.matmul(pv[0:L, 0:128], cT[j][:], vwT[j][:, cs],
                                     start=(j == 0), stop=(j == KT - 1))
                nc.vector.tensor_add(v_sb[:, cs], pv[0:L, 0:128], vb_bc[:, cs])

            # ---------- kq = Wq_h^T (k_h + kb), fp8 x WS ----------
            kq = [[sbb.tile([128, 2, LP], f8, tag=f"kq{j}_{h}",
                            name=f"kq{j}_{h}") for h in range(NH)]
                  for j in range(NJ)]
            for h in range(NH):
                t_, off = h // 2, 64 * (h % 2)
                for j in range(NJ):
                    pkq = ps_o.tile([128, 2, LP], f32, tag="o")
                    for i in range(2):
                        ct = 2 * j + i
                        nc.tensor.matmul(
                            pkq[:, i, :],
                            qw_sb[t_][off:off + 64,
                                      128 * ct:128 * (ct + 1)],
                            kT[t_][off:off + 64, :], start=True, stop=True)
                    nc.scalar.activation(kq[j][h][:], pkq[:], AF.Identity,
                                         scale=WS)
            # qb^T k per head -> exp bias (x attention scale)
            qbk_ps = ps_sm.tile([L, NH], f32, tag="sm")
            for h in range(NH):
                t_, off = h // 2, 64 * (h % 2)
                nc.tensor.matmul(qbk_ps[:, h:h + 1],
                                 kT[t_][off:off + 64, 0:L],
                                 qb_bf[off:off + 64, t_:t_ + 1],
                                 start=True, stop=True)
            qbk = sbb.tile([L, NH], f32, tag="qbk")
            nc.scalar.activation(qbk[:], qbk_ps[:], AF.Identity,
                                 scale=scale_col[0:L, :])

            # ---------- hw-chunk pipeline ----------
            for cix in range(nch):
                hf = (CH * cix) // HLF
                csh = slice(CH * cix - HLF * hf, CH * (cix + 1) - HLF * hf)
                # groupnorm apply -> fp8 pair tiles (Pool engine)
                xn = [sb2.tile([128, 2, CH], f8, tag=f"xn{j}", name=f"xn{j}")
                      for j in range(NJ)]
                for j in range(NJ):
                    for i in range(2):
                        t = 2 * j + i
                        nc.gpsimd.tensor_scalar(
                            xn[j][:, i, :], xb[t][hf][:, csh],
                            scb[:, t:t + 1], scb[:, 4 + t:5 + t],
                            AO.mult, AO.add)
                # logits^T = kq_h^T xn (fp8 DoubleRow) -> exp
                eh = [sb2.tile([L, CH], bf16, tag=f"eh{h}", name=f"eh{h}")
                      for h in range(NH)]
                for h in range(NH):
                    pqk = ps_qk.tile([128, CH], f32, tag="qk")
                    for j in range(NJ):
                        nc.tensor.matmul(
                            pqk[0:LP, :], kq[j][h][:], xn[j][:],
                            start=(j == 0), stop=(j == NJ - 1),
                            perf_mode=mybir.MatmulPerfMode.DoubleRow)
                    nc.scalar.activation(eh[h][:], pqk[0:L, :], AF.Exp,
                                         bias=qbk[:, h:h + 1],
                                         scale=se_col[0:L, :])
                # AV (pair-packed) + replicated sums + normalize -> fp8 pairs
                prj = [sb2.tile([128, 2, CH], f8, tag=f"pr{j}", name=f"pr{j}")
                       for j in range(NJ)]
                for p in range(NT):
                    psm = ps_sm.tile([128, CH], f32, tag="sm")
                    pav = ps_av.tile([128, CH], f32, tag="av")
                    for h in (2 * p, 2 * p + 1):
                        off = 64 * (h % 2)
                        nc.tensor.matmul(psm[off:off + 64, :], ones77[:],
                                         eh[h][:], start=True, stop=True)
                        nc.tensor.matmul(pav[off:off + 64, :],
                                         v_sb[:, 64 * h:64 * h + 64], eh[h][:],
                                         start=True, stop=True)
                    rcp = sb2.tile([128, CH], f32, tag=f"rcp{p % 2}",
                                   name=f"rcp{p}")
                    nc.vector.reciprocal_approx_fast(rcp[:], psm[:])
                    nc.vector.tensor_mul(prj[p // 2][:, p % 2, :], pav[:],
                                         rcp[:])
                # out proj (fp8 DoubleRow) + drain(descale+bias) + residual
                osb = sb3.tile([128, NT, CH], f32, tag="osb")
                res = sb3.tile([128, NT, CH], f32, tag="res")
                for m in range(NT):
                    ms = slice(128 * m, 128 * (m + 1))
                    po = ps_o.tile([128, CH], f32, tag="o")
                    for j in range(NJ):
                        nc.tensor.matmul(
                            po[:], pwdr[j][:, :, ms], prj[j][:],
                            start=(j == 0), stop=(j == NJ - 1),
                            perf_mode=mybir.MatmulPerfMode.DoubleRow)
                    nc.scalar.activation(osb[:, m, :], po[:], AF.Identity,
                                         scale=1.0 / (WS * VS),
                                         bias=colv[:, 16 + m:17 + m])
                    nc.gpsimd.tensor_add(res[:, m, :], osb[:, m, :],
                                         xb[m][hf][:, csh])
                nc.sync.dma_start(out_d[b, :, :, CH * cix:CH * (cix + 1)],
                                  res[:])
        if rep_ctx is not None:
            rep_ctx.__exit__(None, None, None)

    nc.compile()
    return nc


_NC_CACHE = None


def _get_nc():
    global _NC_CACHE
    if _NC_CACHE is None:
        _NC_CACHE = _build_nc()
    return _NC_CACHE


def make_in_maps(x, cond, gamma, beta, q_w, q_b, k_w, k_b, v_w, v_b,
                 proj_w, proj_b, scale):
    x = np.asarray(x, np.float32).reshape(B, C, HW)
    condT = np.asarray(cond, np.float32).transpose(0, 2, 1).astype(BF16)
    qw = np.asarray(q_w, np.float32).astype(BF16)
    kwT = np.ascontiguousarray(np.asarray(k_w, np.float32).T).astype(BF16)
    vwT = np.ascontiguousarray(
        np.asarray(v_w, np.float32).T * VS).astype(BF16)
    pwT = np.ascontiguousarray(np.asarray(proj_w, np.float32).T) * WS
    # DoubleRow packing: pwdr[j][k, i, m] = pwT[256 j + 128 i + k, m]
    pwdr = np.ascontiguousarray(
        pwT.reshape(NJ, 2, 128, C).transpose(0, 2, 1, 3)).astype(F8)
    colv = np.zeros((128, 20), np.float32)
    for t in range(NT):
        s = slice(128 * t, 128 * (t + 1))
        colv[:, t] = np.asarray(gamma, np.float32)[s]
        colv[:, 4 + t] = np.asarray(beta, np.float32)[s]
        colv[:, 8 + t] = np.asarray(q_b, np.float32)[s]
        colv[:, 12 + t] = np.asarray(k_b, np.float32)[s]
        colv[:, 16 + t] = np.asarray(proj_b, np.float32)[s]
    g16 = np.zeros((128, GPT), np.float32)
    for p in range(128):
        g16[p, p // GS] = 1.0 / GS
    g16T = (np.zeros((128, GPT), np.float32) + np.eye(GPT).repeat(GS, 0)).T
    g16T = np.ascontiguousarray(g16T)
    com = dict(
        qw=qw, kwT=kwT, vwT=vwT, pwdr=pwdr, colv=colv,
        vb=np.asarray(v_b, np.float32).reshape(1, C) * VS,
        scale=np.asarray(scale, np.float32).reshape(1, 1),
        g16=g16, g16T=g16T,
    )
    in_maps = []
    for cix in range(N_CORES):
        bs = slice(B_LOC * cix, B_LOC * (cix + 1))
        m = dict(com)
        m["x"] = np.ascontiguousarray(x[bs])
        m["condT"] = np.ascontiguousarray(condT[bs])
        in_maps.append(m)
    return in_maps


def kernel(x, cond, gamma, beta, q_w, q_b, k_w, k_b, v_w, v_b,
           proj_w, proj_b, scale):
    nc = _get_nc()
    in_maps = make_in_maps(x, cond, gamma, beta, q_w, q_b, k_w, k_b,
                           v_w, v_b, proj_w, proj_b, scale)
    res = run_bass_kernel_spmd(nc, in_maps, core_ids=list(range(N_CORES)))
    # out dram layout [B_LOC, 128, NT, HW]: channel c = 128*m + p
    out = np.concatenate([r["out"] for r in res.results], axis=0)
    out = out.transpose(0, 2, 1, 3).reshape(B, C, H, W)
    return np.ascontiguousarray(out).astype(np.float32)
